# revision 2
# baseline (speedup 1.0000x reference)
"""BlockSparseLinear on 8 TRN2 NeuronCores: out = x @ W^T + bias.

Chain-major 1-level Strassen, 8-way data parallel over batch, bf16.
v3: raw x blocks shipped (8 MB/core), Strassen x-sums built on the DVE
interleaved into chain 1's fold stream; weight slabs alternate between
the scalar and gpsimd DMA queues (so neither starves while x streams);
5-deep weight prefetch; small first chunks so the PE starts ~9us in.

  A11 = x_a[:,K1]  A12 = x_a[:,K2]  A21 = x_b[:,K1]  A22 = x_b[:,K2]
  M1=(A11+A22)(B11+B22) M2=(A21+A22)B11 M3=A11(B12-B22)
  M4=A22(B21-B11)       M5=(A11+A12)B22 M6=(A21-A11)(B11+B12)
  M7=(A12-A22)(B21+B22)
  C11=M1+M4-M5+M7  C12=M3+M5  C21=M2+M4  C22=M1-M2+M3+M6

Chain order M3,M4,M2,M5,M1,M6,M7: the first chain only needs A11, and
c21/c12 retire early. SBUF x slots (7 x 2MB): a11, a22, s2, s5, s6 are
persistent tags; a21 lands in slot t1 which is later overwritten by
s7=A12-A22, a12 lands in slot t2 later overwritten by s1=A11+A22.

Per (chain, j o-tile): 16 k-tile matmuls accumulate into one of 4
rotating PSUM banks; DVE folds the bank into persistent bf16
accumulators acc[dst][j] (bias on first touch), final fold writes the
bf16 out staging tile which DMAs out mid-kernel.

Layouts (host pre-blocked so every DMA line is contiguous):
  xk   (4, 128, KH*BH)          block as [k_local, k_tile*b]     bf16
                                order A11, A22, A21, A12
  w    (7, 16, 128, 16, 128)    T_(chain c) as [c, j, k_local,
                                k_tile, o_local]                 bf16
  bias (128, 32)                [o_local, j_global]              f32
  out  (32, 128, 2, 512)        [j_global, o_local, b_half, b]   bf16
"""

import ml_dtypes
import numpy as np

import concourse.mybir as mybir
import concourse.tile as tile
from concourse import bacc
from concourse.bass_utils import run_bass_kernel_spmd

NCORES = 8
BATCH, INF, OUTF = 8192, 4096, 4096
B = BATCH // NCORES          # per-core batch (1024)
BH = B // 2                  # Strassen batch half (512)
KH = 16                      # k-tiles per K-half (of 128 features)
JH = 16                      # o-tiles per O-half (of 128 outputs)
JT = OUTF // 128             # 32 o-tiles total

F32 = mybir.dt.float32
BF16 = mybir.dt.bfloat16
ADD = mybir.AluOpType.add
SUB = mybir.AluOpType.subtract

_NC_CACHE = {}

# Processing order: (Strassen T index, [(dst, sign)]). x operand of the
# c-th chain: a11, a22, s2=A21+A22, s5=A11+A12, s1=A11+A22, s6=A21-A11,
# s7=A12-A22.
ORDER = [
    (2, [("c12", +1), ("c22", +1)]),  # M3 = A11 (B12-B22)
    (3, [("c11", +1), ("c21", +1)]),  # M4 = A22 (B21-B11)
    (1, [("c21", +1), ("c22", -1)]),  # M2 = s2  B11
    (4, [("c12", +1), ("c11", -1)]),  # M5 = s5  B22
    (0, [("c11", +1), ("c22", +1)]),  # M1 = s1  (B11+B22)
    (5, [("c22", +1)]),               # M6 = s6  (B11+B12)
    (6, [("c11", +1)]),               # M7 = s7  (B21+B22)
]
# dst -> (o-half, batch-half)
DSTS = {"c11": (0, 0), "c12": (1, 0), "c21": (0, 1), "c22": (1, 1)}
NTERMS = {"c11": 4, "c12": 2, "c21": 2, "c22": 4}
NQ = 4                       # x chunks per block (4 k-tiles each)


def _build_nc():
    if "nc" in _NC_CACHE:
        return _NC_CACHE["nc"]
    nc = bacc.Bacc("TRN2", target_bir_lowering=False, debug=False,
                   num_devices=NCORES)
    x_d = nc.dram_tensor("xk", [4, 128, KH * BH], BF16,
                         kind="ExternalInput")
    w_d = nc.dram_tensor("w", [7, JH, 128, KH, 128], BF16,
                         kind="ExternalInput")
    b_d = nc.dram_tensor("bias", [128, JT], F32, kind="ExternalInput")
    o_d = nc.dram_tensor("out", [JT, 128, 2, BH], BF16,
                         kind="ExternalOutput")

    with tile.TileContext(nc) as tc:
        with (
            tc.tile_pool(name="xpool", bufs=1) as xpool,
            tc.tile_pool(name="wpool", bufs=5) as wpool,
            tc.tile_pool(name="bpool", bufs=1) as bpool,
            tc.tile_pool(name="apool", bufs=1) as apool,
            tc.tile_pool(name="opool", bufs=2) as opool,
            tc.tile_pool(name="pspool", bufs=4, space="PSUM") as pspool,
        ):
            def xtile(tag):
                return xpool.tile([128, KH, BH], BF16, tag=tag,
                                  name=tag)

            a11, a22 = xtile("a11"), xtile("a22")
            t1a = xtile("t1")   # holds A21 until s2/s6 built
            t2a = xtile("t2")   # holds A12 until s5/s7 built

            # A11 first in 8 small chunks (2 k-tiles) for earliest PE
            # start, then the remaining blocks in 4-k-tile chunks.
            for q in range(8):
                nc.sync.dma_start(
                    out=a11[:, 2 * q:2 * (q + 1), :],
                    in_=x_d[0, :, 2 * q * BH:2 * (q + 1) * BH])
            bias_t = bpool.tile([128, JT], F32, tag="bias", name="bias_t")
            nc.sync.dma_start(out=bias_t[:], in_=b_d[:])
            for blk, t in ((1, a22), (2, t1a), (3, t2a)):
                for q in range(NQ):
                    nc.sync.dma_start(
                        out=t[:, 4 * q:4 * (q + 1), :],
                        in_=x_d[blk, :, 4 * q * BH:4 * (q + 1) * BH])

            s2, s5, s6 = xtile("s2"), xtile("s5"), xtile("s6")
            s7 = xtile("t1")    # reuses A21's slot
            s1 = xtile("t2")    # reuses A12's slot
            xt_op = [a11, a22, s2, s5, s1, s6, s7]

            # DVE sum-build ops, in input-arrival order; interleaved
            # into chain 1's fold stream (DVE is strict FIFO, so they
            # must not get ahead of PSUM-freeing folds).
            sums = []
            for q in range(NQ):
                sl = (slice(None), slice(4 * q, 4 * (q + 1)),
                      slice(None))
                sums.append((s2, sl, t1a, a22, ADD))   # A21+A22
                sums.append((s6, sl, t1a, a11, SUB))   # A21-A11
                sums.append((s5, sl, a11, t2a, ADD))   # A11+A12
            for q in range(NQ):
                sl = (slice(None), slice(4 * q, 4 * (q + 1)),
                      slice(None))
                sums.append((s7, sl, t2a, a22, SUB))   # A12-A22
            for q in range(NQ):
                sl = (slice(None), slice(4 * q, 4 * (q + 1)),
                      slice(None))
                sums.append((s1, sl, a11, a22, ADD))   # A11+A22

            acc = {}
            done = {d: 0 for d in NTERMS}
            nw = 0
            for c, (ti, dsts) in enumerate(ORDER):
                for j in range(JH):
                    wt = wpool.tile([128, KH, 128], BF16, tag="w",
                                    name=f"w_{c}_{j}")
                    eng = nc.scalar if nw % 2 == 0 else nc.gpsimd
                    if nw == 0:
                        # split the very first slab across both queues
                        nc.scalar.dma_start(out=wt[:, :8, :],
                                            in_=w_d[c, j, :, :8])
                        nc.gpsimd.dma_start(out=wt[:, 8:, :],
                                            in_=w_d[c, j, :, 8:])
                    else:
                        eng.dma_start(out=wt[:], in_=w_d[c, j])
                    nw += 1
                    ps = pspool.tile([128, BH], F32, tag="ps",
                                     name=f"ps_{c}_{j}")
                    for k in range(KH):
                        nc.tensor.matmul(
                            ps[:], wt[:, k, :], xt_op[c][:, k, :],
                            start=(k == 0), stop=(k == KH - 1),
                        )
                    for dst, sign in dsts:
                        ohalf, bhalf = DSTS[dst]
                        jg = 16 * ohalf + j
                        final = done[dst] // JH + 1 == NTERMS[dst]
                        key = (dst, j)
                        if key not in acc:
                            assert sign > 0
                            a = apool.tile([128, BH], BF16,
                                           tag=f"a_{dst}_{j}",
                                           name=f"a_{dst}_{j}")
                            nc.vector.tensor_scalar_add(
                                a[:], ps[:], bias_t[:, jg:jg + 1])
                            acc[key] = a
                        elif final:
                            ob = opool.tile([128, BH], BF16, tag="ob",
                                            name=f"ob_{dst}_{j}")
                            nc.vector.tensor_tensor(
                                ob[:], acc[key][:], ps[:],
                                ADD if sign > 0 else SUB,
                            )
                            nc.sync.dma_start(
                                out=o_d[jg, :, bhalf, :], in_=ob[:])
                        else:
                            nc.vector.tensor_tensor(
                                acc[key][:], acc[key][:], ps[:],
                                ADD if sign > 0 else SUB,
                            )
                        done[dst] += 1
                    # drip the x-sum builds into chain 1's idle DVE
                    # slots (all raw blocks have landed by then)
                    if c == 1:
                        for _ in range(2):
                            if sums:
                                dt, sl, in0, in1, op = sums.pop(0)
                                nc.vector.tensor_tensor(
                                    dt[sl], in0[sl], in1[sl], op)
            assert not sums

    nc.compile()
    _NC_CACHE["nc"] = nc
    return nc


def kernel(x, weight, bias):
    x = np.asarray(x, dtype=np.float32)
    weight = np.asarray(weight, dtype=np.float32)
    bias = np.asarray(bias, dtype=np.float32)

    nc = _build_nc()

    # host-side Strassen weight sums (fp32 exact, single bf16 rounding)
    WT = weight.T  # [K, O]
    K1, K2 = slice(0, 2048), slice(2048, 4096)
    O1, O2 = slice(0, 2048), slice(2048, 4096)
    B11, B12 = WT[K1, O1], WT[K1, O2]
    B21, B22 = WT[K2, O1], WT[K2, O2]
    Ts = [B11 + B22, B11, B12 - B22, B21 - B11, B22, B11 + B12,
          B21 + B22]
    wr = np.empty((7, JH, 128, KH, 128), dtype=ml_dtypes.bfloat16)
    for c, (ti, _) in enumerate(ORDER):
        # [2048 K, 2048 O] -> [j, k_local, k_tile, o_local]
        wr[c] = (Ts[ti].astype(ml_dtypes.bfloat16)
                 .reshape(KH, 128, JH, 128).transpose(2, 1, 0, 3))
    br = np.ascontiguousarray(bias.reshape(JT, 128).T)

    in_maps = []
    for cc in range(NCORES):
        xc = x[cc * B:(cc + 1) * B]
        blocks = [xc[0:BH, K1], xc[BH:B, K2], xc[BH:B, K1],
                  xc[0:BH, K2]]  # A11, A22, A21, A12
        xb = np.empty((4, 128, KH * BH), dtype=ml_dtypes.bfloat16)
        for bi, blk in enumerate(blocks):
            # [512 b, 2048 k] -> X=blk.T [k, b] -> [k_local, k_tile*b]
            xb[bi] = (blk.T.astype(ml_dtypes.bfloat16)
                      .reshape(KH, 128, BH).transpose(1, 0, 2)
                      .reshape(128, KH * BH))
        in_maps.append({"xk": xb, "w": wr, "bias": br})

    res = run_bass_kernel_spmd(nc, in_maps, list(range(NCORES)))

    out = np.empty((BATCH, OUTF), np.float32)
    for cc in range(NCORES):
        # [jg, o_local, b_half, b] -> [b_half*b, jg*o_local]
        arr = res.results[cc]["out"].astype(np.float32)
        out[cc * B:(cc + 1) * B] = (arr.transpose(2, 3, 0, 1)
                                    .reshape(B, OUTF))
    return out


# revision 3
# speedup vs baseline: 1.0091x; 1.0091x over previous
"""BlockSparseLinear on 8 TRN2 NeuronCores: out = x @ W^T + bias.

Chain-major 1-level Strassen, 8-way data parallel over batch, bf16.
Schedule: A11 bursts across all three DMA queues at the head so the
first chain can start immediately; weight slabs stream on the scalar
and gpsimd HWDGE queues alternating (a single queue starves during the
x-load window -- SDMA round-robin is per-queue) with a 7-slab
prefetch; the remaining x blocks ride the sync queue; final folds
write in place and DMA straight from the accumulator.

  A11 = x_a[:,K1]  A12 = x_a[:,K2]  A21 = x_b[:,K1]  A22 = x_b[:,K2]
  M1=(A11+A22)(B11+B22) M2=(A21+A22)B11 M3=A11(B12-B22)
  M4=A22(B21-B11)       M5=(A11+A12)B22 M6=(A21-A11)(B11+B12)
  M7=(A12-A22)(B21+B22)
  C11=M1+M4-M5+M7  C12=M3+M5  C21=M2+M4  C22=M1-M2+M3+M6

Chain order M3,M4,M2,M5,M1,M6,M7. SBUF x slots: a11, a22, s2, s5, s6
persistent; a21 lands in slot t1 (later overwritten by s7=A12-A22),
a12 in t2 (later s1=A11+A22). The 5 Strassen x-sums are built on the
DVE, dripped into chain 1's fold stream (strict-FIFO DVE must not
stall PSUM-freeing folds).

Layouts (host pre-blocked so every DMA line is contiguous):
  xk   (4, 128, KH*BH)          block as [k_local, k_tile*b]     bf16
                                order A11, A22, A21, A12
  w    (7, 16, 128, 16, 128)    T_(chain c) as [c, j, k_local,
                                k_tile, o_local]                 bf16
  bias (128, 32)                [o_local, j_global]              f32
  out  (32, 128, 2, 512)        [j_global, o_local, b_half, b]   bf16
"""

import ml_dtypes
import numpy as np

import concourse.mybir as mybir
import concourse.tile as tile
from concourse import bacc
from concourse.bass_utils import run_bass_kernel_spmd

NCORES = 8
BATCH, INF, OUTF = 8192, 4096, 4096
B = BATCH // NCORES          # per-core batch (1024)
BH = B // 2                  # Strassen batch half (512)
KH = 16                      # k-tiles per K-half (of 128 features)
JH = 16                      # o-tiles per O-half (of 128 outputs)
JT = OUTF // 128             # 32 o-tiles total

F32 = mybir.dt.float32
BF16 = mybir.dt.bfloat16
ADD = mybir.AluOpType.add
SUB = mybir.AluOpType.subtract

_NC_CACHE = {}

# Processing order: (Strassen T index, [(dst, sign)]). x operand of the
# c-th chain: a11, a22, s2=A21+A22, s5=A11+A12, s1=A11+A22, s6=A21-A11,
# s7=A12-A22.
ORDER = [
    (2, [("c12", +1), ("c22", +1)]),  # M3 = A11 (B12-B22)
    (3, [("c11", +1), ("c21", +1)]),  # M4 = A22 (B21-B11)
    (1, [("c21", +1), ("c22", -1)]),  # M2 = s2  B11
    (4, [("c12", +1), ("c11", -1)]),  # M5 = s5  B22
    (0, [("c11", +1), ("c22", +1)]),  # M1 = s1  (B11+B22)
    (5, [("c22", +1)]),               # M6 = s6  (B11+B12)
    (6, [("c11", +1)]),               # M7 = s7  (B21+B22)
]
# dst -> (o-half, batch-half)
DSTS = {"c11": (0, 0), "c12": (1, 0), "c21": (0, 1), "c22": (1, 1)}
NTERMS = {"c11": 4, "c12": 2, "c21": 2, "c22": 4}


def _build_nc():
    if "nc" in _NC_CACHE:
        return _NC_CACHE["nc"]
    nc = bacc.Bacc("TRN2", target_bir_lowering=False, debug=False,
                   num_devices=NCORES)
    x_d = nc.dram_tensor("xk", [4, 128, KH * BH], BF16,
                         kind="ExternalInput")
    w_d = nc.dram_tensor("w", [7, JH, 128, KH, 128], BF16,
                         kind="ExternalInput")
    b_d = nc.dram_tensor("bias", [128, JT], F32, kind="ExternalInput")
    o_d = nc.dram_tensor("out", [JT, 128, 2, BH], BF16,
                         kind="ExternalOutput")

    with tile.TileContext(nc) as tc:
        with (
            tc.tile_pool(name="xpool", bufs=1) as xpool,
            tc.tile_pool(name="wpool", bufs=7) as wpool,
            tc.tile_pool(name="bpool", bufs=1) as bpool,
            tc.tile_pool(name="apool", bufs=1) as apool,
            tc.tile_pool(name="pspool", bufs=4, space="PSUM") as pspool,
        ):
            def xtile(tag):
                return xpool.tile([128, KH, BH], BF16, tag=tag,
                                  name=tag)

            a11, a22 = xtile("a11"), xtile("a22")
            t1a = xtile("t1")   # holds A21 until s2/s6 built
            t2a = xtile("t2")   # holds A12 until s5/s7 built

            # A11 bursts across all three queues at the head so it
            # lands at full wire rate (~6us): k0..k5 on sync in small
            # chunks (first matmuls fire ASAP), k6..k10 on scalar,
            # k11..k15 on gpsimd ahead of the weight slabs.
            for k0, nk in [(0, 1), (1, 1), (2, 2), (4, 2)]:
                nc.sync.dma_start(
                    out=a11[:, k0:k0 + nk, :],
                    in_=x_d[0, :, k0 * BH:(k0 + nk) * BH])
            nc.scalar.dma_start(out=a11[:, 6:11, :],
                                in_=x_d[0, :, 6 * BH:11 * BH])
            nc.gpsimd.dma_start(out=a11[:, 11:16, :],
                                in_=x_d[0, :, 11 * BH:16 * BH])
            bias_t = bpool.tile([128, JT], F32, tag="bias", name="bias_t")
            nc.sync.dma_start(out=bias_t[:], in_=b_d[:])
            # remaining raw blocks on the sync queue (needed from
            # t~68us on; sync drains them while the weight queues run)
            for blk, t in ((1, a22), (2, t1a), (3, t2a)):
                for q in range(4):
                    nc.sync.dma_start(
                        out=t[:, 4 * q:4 * (q + 1), :],
                        in_=x_d[blk, :, 4 * q * BH:4 * (q + 1) * BH])

            s2, s5, s6 = xtile("s2"), xtile("s5"), xtile("s6")
            s7 = xtile("t1")    # reuses A21's slot
            s1 = xtile("t2")    # reuses A12's slot
            xt_op = [a11, a22, s2, s5, s1, s6, s7]

            # DVE sum-build ops in input-arrival order; dripped into
            # chain 1's fold stream.
            sums = []
            for q in range(4):
                sl = (slice(None), slice(4 * q, 4 * (q + 1)),
                      slice(None))
                sums.append((s2, sl, t1a, a22, ADD))   # A21+A22
                sums.append((s6, sl, t1a, a11, SUB))   # A21-A11
                sums.append((s5, sl, a11, t2a, ADD))   # A11+A12
            for q in range(4):
                sl = (slice(None), slice(4 * q, 4 * (q + 1)),
                      slice(None))
                sums.append((s7, sl, t2a, a22, SUB))   # A12-A22
            for q in range(4):
                sl = (slice(None), slice(4 * q, 4 * (q + 1)),
                      slice(None))
                sums.append((s1, sl, a11, a22, ADD))   # A11+A22

            acc = {}
            done = {d: 0 for d in NTERMS}
            nw = 0
            for c, (ti, dsts) in enumerate(ORDER):
                for j in range(JH):
                    wt = wpool.tile([128, KH, 128], BF16, tag="w",
                                    name=f"w_{c}_{j}")
                    if nw < 3:
                        # split the first slabs across both queues
                        nc.scalar.dma_start(out=wt[:, :8, :],
                                            in_=w_d[c, j, :, :8])
                        nc.gpsimd.dma_start(out=wt[:, 8:, :],
                                            in_=w_d[c, j, :, 8:])
                    elif nw % 2 == 0:
                        nc.scalar.dma_start(out=wt[:], in_=w_d[c, j])
                    else:
                        nc.gpsimd.dma_start(out=wt[:], in_=w_d[c, j])
                    nw += 1
                    ps = pspool.tile([128, BH], F32, tag="ps",
                                     name=f"ps_{c}_{j}")
                    for k in range(KH):
                        nc.tensor.matmul(
                            ps[:], wt[:, k, :], xt_op[c][:, k, :],
                            start=(k == 0), stop=(k == KH - 1),
                        )
                    for dst, sign in dsts:
                        ohalf, bhalf = DSTS[dst]
                        jg = 16 * ohalf + j
                        final = done[dst] // JH + 1 == NTERMS[dst]
                        key = (dst, j)
                        if key not in acc:
                            assert sign > 0
                            a = apool.tile([128, BH], BF16,
                                           tag=f"a_{dst}_{j}",
                                           name=f"a_{dst}_{j}")
                            nc.vector.tensor_scalar_add(
                                a[:], ps[:], bias_t[:, jg:jg + 1])
                            acc[key] = a
                        else:
                            nc.vector.tensor_tensor(
                                acc[key][:], acc[key][:], ps[:],
                                ADD if sign > 0 else SUB,
                            )
                            if final:
                                nc.sync.dma_start(
                                    out=o_d[jg, :, bhalf, :],
                                    in_=acc[key][:])
                        done[dst] += 1
                    # drip the x-sum builds into chain 1's idle DVE
                    # slots (all raw blocks have landed by then)
                    if c == 1:
                        for _ in range(2):
                            if sums:
                                dt, sl, in0, in1, op = sums.pop(0)
                                nc.vector.tensor_tensor(
                                    dt[sl], in0[sl], in1[sl], op)
            assert not sums

    nc.compile()
    _NC_CACHE["nc"] = nc
    return nc


def kernel(x, weight, bias):
    x = np.asarray(x, dtype=np.float32)
    weight = np.asarray(weight, dtype=np.float32)
    bias = np.asarray(bias, dtype=np.float32)

    nc = _build_nc()

    # host-side Strassen weight sums (fp32 exact, single bf16 rounding)
    WT = weight.T  # [K, O]
    K1, K2 = slice(0, 2048), slice(2048, 4096)
    O1, O2 = slice(0, 2048), slice(2048, 4096)
    B11, B12 = WT[K1, O1], WT[K1, O2]
    B21, B22 = WT[K2, O1], WT[K2, O2]
    Ts = [B11 + B22, B11, B12 - B22, B21 - B11, B22, B11 + B12,
          B21 + B22]
    wr = np.empty((7, JH, 128, KH, 128), dtype=ml_dtypes.bfloat16)
    for c, (ti, _) in enumerate(ORDER):
        # [2048 K, 2048 O] -> [j, k_local, k_tile, o_local]
        wr[c] = (Ts[ti].astype(ml_dtypes.bfloat16)
                 .reshape(KH, 128, JH, 128).transpose(2, 1, 0, 3))
    br = np.ascontiguousarray(bias.reshape(JT, 128).T)

    in_maps = []
    for cc in range(NCORES):
        xc = x[cc * B:(cc + 1) * B]
        blocks = [xc[0:BH, K1], xc[BH:B, K2], xc[BH:B, K1],
                  xc[0:BH, K2]]  # A11, A22, A21, A12
        xb = np.empty((4, 128, KH * BH), dtype=ml_dtypes.bfloat16)
        for bi, blk in enumerate(blocks):
            # [512 b, 2048 k] -> X=blk.T [k, b] -> [k_local, k_tile*b]
            xb[bi] = (blk.T.astype(ml_dtypes.bfloat16)
                      .reshape(KH, 128, BH).transpose(1, 0, 2)
                      .reshape(128, KH * BH))
        in_maps.append({"xk": xb, "w": wr, "bias": br})

    res = run_bass_kernel_spmd(nc, in_maps, list(range(NCORES)))

    out = np.empty((BATCH, OUTF), np.float32)
    for cc in range(NCORES):
        # [jg, o_local, b_half, b] -> [b_half*b, jg*o_local]
        arr = res.results[cc]["out"].astype(np.float32)
        out[cc * B:(cc + 1) * B] = (arr.transpose(2, 3, 0, 1)
                                    .reshape(B, OUTF))
    return out
